# revision 28
# baseline (speedup 1.0000x reference)
"""Trainium2 Bass kernel for nn_AttentionHead (single-head attention with RoPE + QK-norm).

Contract: kernel(**inputs) takes FULL unsharded inputs
  input_vecs [4, 2048, 1024] f32, qkv_w [3072, 1024] f32, sqk [1024] f32
and returns the FULL output [4, 2048, 1024] f32.

Sharding: 8 cores = (batch b, half z). Core (b, z):
  - projects K and V only for ITS half of the sequence (tokens [z*1024, z*1024+1024));
    the pair (2b, 2b+1) exchanges K/V (+ row norms) via pairwise AllGather
    collectives, removing the K/V projection redundancy;
  - computes attention for batch b's query blocks {0,3} (z=0) or {1,2} (z=1)
    (512-token blocks) -- a balanced split of causal attention work.

On-chip layout: no transposes anywhere. Q^T/K^T come out of the projection
matmuls directly in [d, t] layout; V in natural [t, d]. Scores are computed
transposed [k, q] so softmax denominators are tiny ones-matmuls and the exp
tile is directly the lhsT of the attention*V matmul. Heavy matmuls are bf16
with fp32 PSUM accumulate. RoPE is fused with the per-channel sqk^2 scale
(scalar_tensor_tensor); 1/||K|| folds into the exp's per-partition scale;
1/||Q|| applies via a rank-1 broadcast matmul.
"""

import numpy as np
import ml_dtypes

D = 1024          # d_model == d_internal
T = 2048
B = 4
HALF = T // 2     # 1024 tokens of K/V per core
ROPE_BASE = 10000.0
NC = 8            # cores
ND = D // 128     # 8 d-tiles
NTC = D // 128    # 8 contraction c-tiles
QB = 512          # query block size
NKLO, NKHI = 8, 16  # k-tiles processed for chunk-lo / chunk-hi (uniform program)

BF16 = ml_dtypes.bfloat16

# ---------------------------------------------------------------------------
# Infra patch: this walrus build supports only ONE sync-wait per instruction.
# Tile attaches multiple; split the extras onto NoOps inserted just before.
# ---------------------------------------------------------------------------
_PATCHED = False


def _install_patches():
    global _PATCHED
    if _PATCHED:
        return
    _PATCHED = True
    import json as _json
    import concourse.bass as _bass

    orig = _bass.Bass.to_json_bytes

    def _split_waits(m):
        ctr = 0
        for fn in m.get("functions", []):
            for blk in fn.get("blocks", []):
                insts = blk.get("instructions")
                if not insts:
                    continue
                out = []
                changed = False
                for inst in insts:
                    si = inst.get("sync_info")
                    w = (si or {}).get("on_wait") or []
                    if len(w) > 1:
                        changed = True
                        for j in range(len(w) - 1):
                            ctr += 1
                            out.append({
                                "name": f"waitsplit-{ctr}-{inst['name']}",
                                "engine": inst["engine"],
                                "opcode": "NoOp",
                                "ins": [],
                                "outs": [],
                                "sync_info": {"on_wait": [w[j]], "on_update": []},
                            })
                        si["on_wait"] = [w[-1]]
                    out.append(inst)
                if changed:
                    blk["instructions"] = out
        return m, ctr

    def to_json_bytes(self):
        raw = orig(self)
        m = _json.loads(raw)
        m, n = _split_waits(m)
        if n:
            raw = _json.dumps(m).encode()
        return raw

    _bass.Bass.to_json_bytes = to_json_bytes


# ---------------------------------------------------------------------------
# Bass program (identical for all 8 cores; per-core behavior comes from data)
# ---------------------------------------------------------------------------
_PROGRAM = None
_GROUPS = [[0, 1], [2, 3], [4, 5], [6, 7]]


def _build_program():
    import concourse.bass as bass
    import concourse.mybir as mybir
    from concourse.tile import TileContext

    BF = mybir.dt.bfloat16
    F32 = mybir.dt.float32
    AF = mybir.ActivationFunctionType
    OP = mybir.AluOpType

    nc = bass.Bass(num_devices=NC)

    # ---- I/O ----
    xh_d = nc.dram_tensor("xh", [D, HALF], BF, kind="ExternalInput")       # x^T cols of my half
    xq_d = nc.dram_tensor("xq", [D, 2 * QB], BF, kind="ExternalInput")     # x^T cols of q-blocks
    wt_d = nc.dram_tensor("wt", [D, 3 * D], BF, kind="ExternalInput")      # W^T
    cosh_d = nc.dram_tensor("cosh", [D // 2, HALF], BF, kind="ExternalInput")
    sinh_d = nc.dram_tensor("sinh", [D // 2, HALF], BF, kind="ExternalInput")
    cosq_d = nc.dram_tensor("cosq", [D // 2, 2 * QB], BF, kind="ExternalInput")
    sinq_d = nc.dram_tensor("sinq", [D // 2, 2 * QB], BF, kind="ExternalInput")
    s2_d = nc.dram_tensor("s2", [D, 1], F32, kind="ExternalInput")         # 32 * (32*sqk)^2
    mlo_d = nc.dram_tensor("masklo", [NKLO, 128, QB], BF, kind="ExternalInput")
    mhi_d = nc.dram_tensor("maskhi", [NKHI - NKLO, 128, QB], BF, kind="ExternalInput")
    out_d = nc.dram_tensor("out", [2 * QB, D], F32, kind="ExternalOutput")
    # collective staging (Local internal DRAM)
    kh_d = nc.dram_tensor("khalf", [D, HALF], BF, kind="Internal")         # my K^T half
    kg_d = nc.dram_tensor("kgath", [2 * D, HALF], BF, kind="Internal")     # [rank0 K^T; rank1 K^T]
    vh_d = nc.dram_tensor("vhalf", [HALF, D], BF, kind="Internal")
    vg_d = nc.dram_tensor("vgath", [T, D], BF, kind="Internal")            # global V
    rh_d = nc.dram_tensor("rnkh", [128, 8], F32, kind="Internal")
    rg_d = nc.dram_tensor("rnkg", [256, 8], F32, kind="Internal")

    with TileContext(nc) as tc:
        with tc.tile_pool(name="persist", bufs=1) as pp:
            # persistent SBUF tiles (KB/partition in comments)
            xh = pp.tile([128, NTC * HALF], BF, tag="xh")         # 16K
            xq = pp.tile([128, NTC * 2 * QB], BF, tag="xq")       # 16K
            wqk = pp.tile([128, NTC * 2 * D], BF, tag="wqk")      # 32K (W_q|W_k cols)
            wv = pp.tile([128, NTC * D], BF, tag="wv")            # 16K
            qt = pp.tile([128, ND * 2 * QB], BF, tag="qt")        # 16K
            kt = pp.tile([128, ND * T], BF, tag="kt")             # 32K (global K^T, post-gather)
            s2 = pp.tile([128, ND], F32, tag="s2")
            rnk = pp.tile([128, 16], F32, tag="rnk")              # 1/||K row||, per global k-tile
            ones_bf = pp.tile([128, 1], BF, tag="ones_bf")
            ones1x = pp.tile([1, 128], F32, tag="ones1x")

            # ---- P0: load resident data (K-projection inputs first) ----
            for i in range(ND):
                nc.sync.dma_start(wqk[:, i * 2 * D:(i + 1) * 2 * D], wt_d[i * 128:(i + 1) * 128, 0:2 * D])
                nc.sync.dma_start(xh[:, i * HALF:(i + 1) * HALF], xh_d[i * 128:(i + 1) * 128, :])
                nc.sync.dma_start(s2[:, i:i + 1], s2_d[i * 128:(i + 1) * 128, :])
            for i in range(ND):
                nc.sync.dma_start(xq[:, i * 2 * QB:(i + 1) * 2 * QB], xq_d[i * 128:(i + 1) * 128, :])
                nc.sync.dma_start(wv[:, i * D:(i + 1) * D], wt_d[i * 128:(i + 1) * 128, 2 * D:3 * D])
            nc.vector.memset(ones_bf[:], 1.0)
            nc.vector.memset(ones1x[:], 1.0)

            # ---- P2: K projection (my half) + rope + s2-fold + row norms ----
            # kth: [d, k_local] staging; then DMA to DRAM and pairwise-AllGather.
            with tc.tile_pool(name="p2sb", bufs=1) as sp2, \
                 tc.tile_pool(name="p2cs", bufs=2) as cspool2, \
                 tc.tile_pool(name="p2km", bufs=18) as kmpool, \
                 tc.tile_pool(name="p2scr", bufs=3) as scr2, \
                 tc.tile_pool(name="p2ps", bufs=2, space="PSUM") as psp2, \
                 tc.tile_pool(name="p2psn", bufs=1, space="PSUM") as psn2:
                kth = sp2.tile([128, ND * HALF], BF, tag="kth")   # 16K
                rnkh = sp2.tile([128, 8], F32, tag="rnkh")
                pending = []

                def emit_norms(ch, ktmps):
                    pnk0 = psn2.tile([128, 1], F32, tag="pnk0")
                    pnk1 = psn2.tile([128, 1], F32, tag="pnk1")
                    pnk2 = psn2.tile([128, 1], F32, tag="pnk2")
                    pnk3 = psn2.tile([128, 1], F32, tag="pnk3")
                    pnks = (pnk0, pnk1, pnk2, pnk3)
                    for idx, km in enumerate(ktmps):
                        sqk_t = scr2.tile([128, QB], BF, tag="sqk")
                        nc.vector.tensor_tensor(sqk_t[:], km[:], km[:], op=OP.mult)
                        for sub in range(4):
                            nc.tensor.matmul(pnks[sub][:],
                                             sqk_t[:, sub * 128:(sub + 1) * 128], ones_bf[:],
                                             start=(idx == 0), stop=(idx == 7))
                    for sub in range(4):
                        rtmp = scr2.tile([128, 1], F32, tag="rtmp")
                        nc.scalar.activation(rtmp[:], pnks[sub][:], AF.Sqrt)
                        nc.vector.reciprocal(rnkh[:, ch * 4 + sub:ch * 4 + sub + 1], rtmp[:])

                for ch in range(2):
                    k0 = ch * QB
                    cosc = cspool2.tile([128, 4 * QB], BF, tag="cosc")
                    sinc = cspool2.tile([128, 4 * QB], BF, tag="sinc")
                    for i in range(4):
                        nc.sync.dma_start(cosc[:, i * QB:(i + 1) * QB], cosh_d[i * 128:(i + 1) * 128, k0:k0 + QB])
                        nc.sync.dma_start(sinc[:, i * QB:(i + 1) * QB], sinh_d[i * 128:(i + 1) * 128, k0:k0 + QB])
                    # dense projection matmuls; PSUM copied to bf16 SBUF immediately
                    ktmps = []
                    for j in range(4):
                        for dt, ptag in ((j, "pka"), (j + 4, "pkb")):
                            p = psp2.tile([128, QB], F32, tag=ptag)
                            for c in range(NTC):
                                nc.tensor.matmul(p[:], wqk[:, c * 2 * D + D + dt * 128: c * 2 * D + D + (dt + 1) * 128],
                                                 xh[:, c * HALF + k0: c * HALF + k0 + QB],
                                                 start=(c == 0), stop=(c == NTC - 1))
                            km = kmpool.tile([128, QB], BF, tag="ktmp")
                            nc.scalar.copy(km[:], p[:])
                            ktmps.append(km)
                    # rope with fused s2 scale (bf16 in/out for DVE 2x mode)
                    for j in range(4):
                        km_a, km_b = ktmps[2 * j], ktmps[2 * j + 1]
                        ca = cosc[:, j * QB:(j + 1) * QB]
                        sa = sinc[:, j * QB:(j + 1) * QB]
                        t_a = scr2.tile([128, QB], BF, tag="kra")
                        t_b = scr2.tile([128, QB], BF, tag="krb")
                        nc.vector.scalar_tensor_tensor(t_a[:], km_a[:], s2[:, j:j + 1], ca, op0=OP.mult, op1=OP.mult)
                        nc.vector.scalar_tensor_tensor(t_b[:], km_b[:], s2[:, j:j + 1], sa, op0=OP.mult, op1=OP.mult)
                        nc.vector.tensor_sub(kth[:, j * HALF + k0: j * HALF + k0 + QB], t_a[:], t_b[:])
                        t_c = scr2.tile([128, QB], BF, tag="kra")
                        t_e = scr2.tile([128, QB], BF, tag="krb")
                        nc.vector.scalar_tensor_tensor(t_c[:], km_b[:], s2[:, j + 4:j + 5], ca, op0=OP.mult, op1=OP.mult)
                        nc.vector.scalar_tensor_tensor(t_e[:], km_a[:], s2[:, j + 4:j + 5], sa, op0=OP.mult, op1=OP.mult)
                        nc.vector.tensor_add(kth[:, (j + 4) * HALF + k0: (j + 4) * HALF + k0 + QB], t_c[:], t_e[:])
                    pending.append((ch, ktmps))
                    if ch > 0:
                        emit_norms(*pending.pop(0))
                emit_norms(*pending.pop(0))

                # ship my K^T half + norms, AllGather across the pair
                for i in range(ND):
                    nc.sync.dma_start(kh_d[i * 128:(i + 1) * 128, :], kth[:, i * HALF:(i + 1) * HALF])
                nc.sync.dma_start(rh_d[:, :], rnkh[:])
                nc.gpsimd.collective_compute(
                    kind="AllGather", op=OP.bypass, replica_groups=_GROUPS,
                    ins=[kh_d[:, :]], outs=[kg_d[:, :]])
                nc.gpsimd.collective_compute(
                    kind="AllGather", op=OP.bypass, replica_groups=_GROUPS,
                    ins=[rh_d[:, :]], outs=[rg_d[:, :]])
                # reload global K^T and norms (scalar HWDGE queue; Sync stays free for P1)
                for i in range(ND):
                    nc.scalar.dma_start(kt[:, i * T: i * T + HALF], kg_d[i * 128:(i + 1) * 128, :])
                    nc.scalar.dma_start(kt[:, i * T + HALF: (i + 1) * T], kg_d[D + i * 128: D + (i + 1) * 128, :])
                nc.scalar.dma_start(rnk[:, 0:8], rg_d[0:128, :])
                nc.scalar.dma_start(rnk[:, 8:16], rg_d[128:256, :])

            # ---- P1: Q projection + rope + normalize -> qt (bf16, [d, q]) ----
            with tc.tile_pool(name="p1sb", bufs=2) as sp1, \
                 tc.tile_pool(name="p1cs", bufs=1) as cspool1, \
                 tc.tile_pool(name="p1scr", bufs=3) as scr1, \
                 tc.tile_pool(name="p1ps", bufs=4, space="PSUM") as psp1, \
                 tc.tile_pool(name="p1psn", bufs=2, space="PSUM") as psn1:
                cosq = cspool1.tile([128, 4 * 2 * QB], BF, tag="cosq")    # 8K
                sinq = cspool1.tile([128, 4 * 2 * QB], BF, tag="sinq")    # 8K
                for i in range(4):
                    nc.sync.dma_start(cosq[:, i * 2 * QB:(i + 1) * 2 * QB], cosq_d[i * 128:(i + 1) * 128, :])
                    nc.sync.dma_start(sinq[:, i * 2 * QB:(i + 1) * 2 * QB], sinq_d[i * 128:(i + 1) * 128, :])
                qtmps = []
                for ch in range(2):
                    q0 = ch * QB
                    qtmp = sp1.tile([128, ND * QB], BF, tag="qtmp")       # 8K (bf16 copy of proj)
                    for i in range(ND):
                        p = psp1.tile([128, QB], F32, tag="pproj")
                        for c in range(NTC):
                            nc.tensor.matmul(p[:], wqk[:, c * 2 * D + i * 128: c * 2 * D + (i + 1) * 128],
                                             xq[:, c * 2 * QB + q0: c * 2 * QB + q0 + QB],
                                             start=(c == 0), stop=(c == NTC - 1))
                        nc.scalar.copy(qtmp[:, i * QB:(i + 1) * QB], p[:])
                    qtmps.append(qtmp)
                for ch in range(2):
                    q0 = ch * QB
                    qtmp = qtmps[ch]
                    pnq = psn1.tile([1, QB], F32, tag="pnq")
                    for i in range(ND):
                        sq = scr1.tile([128, QB], BF, tag="sq")
                        nc.vector.tensor_tensor(sq[:], qtmp[:, i * QB:(i + 1) * QB],
                                                qtmp[:, i * QB:(i + 1) * QB], op=OP.mult)
                        nc.tensor.matmul(pnq[:], ones_bf[:], sq[:], start=(i == 0), stop=(i == ND - 1))
                    # 1/||q row|| as [1, QB], then broadcast to [128, QB] via rank-1 matmul
                    rnq = sp1.tile([1, QB], F32, tag="rnq")
                    nc.scalar.activation(rnq[:], pnq[:], AF.Sqrt)
                    nc.vector.reciprocal(rnq[:], rnq[:])
                    pbc = psn1.tile([128, QB], F32, tag="pbc")
                    nc.tensor.matmul(pbc[:], ones1x[:], rnq[:], start=True, stop=True)
                    for i in range(4):
                        ca = cosq[:, i * 2 * QB + q0: i * 2 * QB + q0 + QB]
                        sa = sinq[:, i * 2 * QB + q0: i * 2 * QB + q0 + QB]
                        lo = qtmp[:, i * QB:(i + 1) * QB]
                        hi = qtmp[:, (i + 4) * QB:(i + 5) * QB]
                        t_a = scr1.tile([128, QB], BF, tag="ropea")
                        t_b = scr1.tile([128, QB], BF, tag="ropeb")
                        # lower half: lo*cos - hi*sin   (then * rnq broadcast)
                        nc.vector.tensor_tensor(t_a[:], lo, ca, op=OP.mult)
                        nc.vector.tensor_tensor(t_b[:], hi, sa, op=OP.mult)
                        nc.vector.tensor_sub(t_a[:], t_a[:], t_b[:])
                        nc.vector.tensor_tensor(qt[:, i * 2 * QB + q0: i * 2 * QB + q0 + QB],
                                                t_a[:], pbc[:], op=OP.mult)
                        # upper half: hi*cos + lo*sin
                        t_c = scr1.tile([128, QB], BF, tag="ropea")
                        t_e = scr1.tile([128, QB], BF, tag="ropeb")
                        nc.vector.tensor_tensor(t_c[:], hi, ca, op=OP.mult)
                        nc.vector.tensor_tensor(t_e[:], lo, sa, op=OP.mult)
                        nc.vector.tensor_add(t_c[:], t_c[:], t_e[:])
                        nc.vector.tensor_tensor(qt[:, (i + 4) * 2 * QB + q0: (i + 4) * 2 * QB + q0 + QB],
                                                t_c[:], pbc[:], op=OP.mult)

            # ---- P3: V projection (my half, natural [t, d]) -> DRAM + AllGather ----
            with tc.tile_pool(name="p3v", bufs=3) as vp3, \
                 tc.tile_pool(name="p3ps", bufs=3, space="PSUM") as psp3:
                for tt in range(HALF // 128):
                    vt = vp3.tile([128, D], BF, tag="vt")
                    for dch in range(2):
                        p = psp3.tile([128, 512], F32, tag="pv")
                        for c in range(NTC):
                            nc.tensor.matmul(p[:], xh[:, c * HALF + tt * 128: c * HALF + (tt + 1) * 128],
                                             wv[:, c * D + dch * 512: c * D + (dch + 1) * 512],
                                             start=(c == 0), stop=(c == NTC - 1))
                        nc.scalar.copy(vt[:, dch * 512:(dch + 1) * 512], p[:])
                    nc.sync.dma_start(vh_d[tt * 128:(tt + 1) * 128, :], vt[:])
                nc.gpsimd.collective_compute(
                    kind="AllGather", op=OP.bypass, replica_groups=_GROUPS,
                    ins=[vh_d[:, :]], outs=[vg_d[:, :]])

            # ---- P4: attention ----
            with tc.tile_pool(name="p4exp", bufs=2) as ep4, \
                 tc.tile_pool(name="p4m", bufs=3) as mp4, \
                 tc.tile_pool(name="p4v", bufs=4) as vp4, \
                 tc.tile_pool(name="p4o", bufs=2) as op4, \
                 tc.tile_pool(name="p4scr", bufs=2) as scr4, \
                 tc.tile_pool(name="p4pss", bufs=2, space="PSUM") as pss, \
                 tc.tile_pool(name="p4psd", bufs=1, space="PSUM") as psd, \
                 tc.tile_pool(name="p4pso", bufs=1, space="PSUM") as pso:
                for ch, (n_k, mask_d, mask_start) in enumerate(
                        ((NKLO, mlo_d, 0), (NKHI, mhi_d, NKLO))):
                    q0 = ch * QB
                    ex = ep4.tile([128, NKHI * QB], BF, tag="exp")        # 16K
                    for kti in range(n_k):
                        ps_s = pss.tile([128, QB], F32, tag="pscore")
                        for i in range(ND):
                            nc.tensor.matmul(ps_s[:], kt[:, i * T + kti * 128: i * T + (kti + 1) * 128],
                                             qt[:, i * 2 * QB + q0: i * 2 * QB + q0 + QB],
                                             start=(i == 0), stop=(i == ND - 1))
                        exsl = ex[:, kti * QB:(kti + 1) * QB]
                        nc.scalar.activation(exsl, ps_s[:], AF.Exp, bias=0.0, scale=rnk[:, kti:kti + 1])
                        if kti >= mask_start:
                            mt = mp4.tile([128, QB], BF, tag="mask")
                            nc.gpsimd.dma_start(mt[:], mask_d[kti - mask_start, :, :])
                            nc.vector.tensor_tensor(exsl, exsl, mt[:], op=OP.mult)
                    # attention * V, two query-subtiles at a time (psum budget).
                    # One accumulation group per PSUM tile. Denominator matmuls are
                    # emitted inside the first sp pass (they only gate the out-copy).
                    rden = None
                    for sp in range(2):
                        poa = pso.tile([128, 512], F32, tag="pout0")
                        pob = pso.tile([128, 512], F32, tag="pout1")
                        poc = pso.tile([128, 512], F32, tag="pout2")
                        pod = pso.tile([128, 512], F32, tag="pout3")
                        po = ((poa, pob), (poc, pod))
                        for kti in range(n_k):
                            vt = vp4.tile([128, D], BF, tag="v4")
                            nc.gpsimd.dma_start(vt[:], vg_d[kti * 128:(kti + 1) * 128, :])
                            for s01 in range(2):
                                sub = sp * 2 + s01
                                for dch in range(2):
                                    nc.tensor.matmul(po[s01][dch][:],
                                                     ex[:, kti * QB + sub * 128: kti * QB + (sub + 1) * 128],
                                                     vt[:, dch * 512:(dch + 1) * 512],
                                                     start=(kti == 0), stop=(kti == n_k - 1))
                        if sp == 0:
                            pden = psd.tile([128, 4], F32, tag="pden")
                            for sub in range(4):
                                for kti in range(n_k):
                                    nc.tensor.matmul(pden[:, sub:sub + 1],
                                                     ex[:, kti * QB + sub * 128: kti * QB + (sub + 1) * 128],
                                                     ones_bf[:], start=(kti == 0), stop=(kti == n_k - 1))
                            rden = scr4.tile([128, 4], F32, tag="rden")
                            nc.vector.reciprocal(rden[:], pden[:])
                        for s01 in range(2):
                            sub = sp * 2 + s01
                            ot = op4.tile([128, D], F32, tag="ot")
                            for dch in range(2):
                                nc.scalar.activation(ot[:, dch * 512:(dch + 1) * 512],
                                                     po[s01][dch][:], AF.Copy, bias=0.0,
                                                     scale=rden[:, sub:sub + 1])
                            nc.sync.dma_start(out_d[q0 + sub * 128: q0 + (sub + 1) * 128, :], ot[:])

    return nc


def _get_program():
    global _PROGRAM
    if _PROGRAM is None:
        _install_patches()
        _PROGRAM = _build_program()
    return _PROGRAM


# ---------------------------------------------------------------------------
# Host-side prep + launch
# ---------------------------------------------------------------------------
def _rope_tables():
    inv_freq = (1.0 / (ROPE_BASE ** (np.arange(0, D, 2, dtype=np.float32) / D))).astype(np.float32)
    t = np.arange(T, dtype=np.float32)
    freqs = t[:, None] * inv_freq[None, :]          # [T, 512]
    cos = np.cos(freqs).T.copy()                    # [512, T]
    sin = np.sin(freqs).T.copy()
    return cos, sin


def _mask_tiles(block, kt_lo, kt_hi):
    """[kt_hi-kt_lo, 128, 512] 0/1: allowed = key_global <= query_global."""
    n = kt_hi - kt_lo
    m = np.zeros((n, 128, QB), dtype=np.float32)
    qg = block * QB + np.arange(QB)[None, :]
    for idx, kti in enumerate(range(kt_lo, kt_hi)):
        kg = kti * 128 + np.arange(128)[:, None]
        m[idx] = (kg <= qg).astype(np.float32)
    return m


# kept for test.py introspection
LAST_RESULT = None


def kernel(input_vecs, qkv_w, sqk, _trace=False):
    global LAST_RESULT
    _install_patches()
    from concourse.bass_utils import run_bass_kernel_spmd

    nc = _get_program()

    f32 = np.float32
    x = np.asarray(input_vecs, f32)
    w = np.asarray(qkv_w, f32)
    s = np.asarray(sqk, f32)

    wt_bf = np.ascontiguousarray(w.T).astype(BF16)                  # [1024, 3072]
    sqk_eff = s * np.sqrt(np.float32(D)).astype(f32)
    s2 = (np.sqrt(np.float32(D)).astype(f32) * sqk_eff * sqk_eff).reshape(D, 1).astype(f32)
    cos, sin = _rope_tables()

    in_maps = []
    metas = []
    for c in range(NC):
        b, z = c // 2, c % 2
        blo, bhi = (0, 3) if z == 0 else (1, 2)
        xt = np.ascontiguousarray(x[b].T)                           # [1024, 2048] f32
        qcols = np.concatenate([xt[:, blo * QB:(blo + 1) * QB],
                                xt[:, bhi * QB:(bhi + 1) * QB]], axis=1)
        cosq = np.concatenate([cos[:, blo * QB:(blo + 1) * QB],
                               cos[:, bhi * QB:(bhi + 1) * QB]], axis=1)
        sinq = np.concatenate([sin[:, blo * QB:(blo + 1) * QB],
                               sin[:, bhi * QB:(bhi + 1) * QB]], axis=1)
        h0 = z * HALF
        in_maps.append({
            "xh": np.ascontiguousarray(xt[:, h0:h0 + HALF]).astype(BF16),
            "xq": np.ascontiguousarray(qcols).astype(BF16),
            "wt": wt_bf,
            "cosh": np.ascontiguousarray(cos[:, h0:h0 + HALF]).astype(BF16),
            "sinh": np.ascontiguousarray(sin[:, h0:h0 + HALF]).astype(BF16),
            "cosq": np.ascontiguousarray(cosq).astype(BF16),
            "sinq": np.ascontiguousarray(sinq).astype(BF16),
            "s2": s2,
            "masklo": _mask_tiles(blo, 0, NKLO).astype(BF16),
            "maskhi": _mask_tiles(bhi, NKLO, NKHI).astype(BF16),
        })
        metas.append((b, blo, bhi))

    res = run_bass_kernel_spmd(nc, in_maps, core_ids=list(range(NC)), trace=_trace)
    LAST_RESULT = res

    out = np.empty((B, T, D), dtype=f32)
    for c, (b, blo, bhi) in enumerate(metas):
        o = np.asarray(res.results[c]["out"], f32)
        out[b, blo * QB:(blo + 1) * QB] = o[:QB]
        out[b, bhi * QB:(bhi + 1) * QB] = o[QB:]
    return out
